# revision 28
# baseline (speedup 1.0000x reference)
"""TRN2 Bass/Tile kernel for nn_BlockSparseMoE (T=2048, D=1024, F=2048, E=8, top-2).

Expert parallelism across the 8 NeuronCores: core c owns expert c. The host
performs routing (top-2 of an [T, E] logit matmul — microseconds of numpy) and
the expert-parallel all-to-all dispatch/combine: it gathers each expert's
tokens into a compact d-major activation block xcT = x[idx_e].T, and after the
device run scatters coef * y back into the full [T, D] output.

The device NEFF is a pure fused GLU FFN per expert, in bf16 (fp32 PSUM
accumulate), sized to the actual max expert load C:

  M12  a = W1 @ xc, b = V1 @ xc  (f-major [128f, C] PSUM chains over 8 d-tiles)
       hT[f] = silu(a) * b  (ACT silu + DVE mult, bf16)
  M3T  yT[d] = sum_f W2[f, d-block]^T-chain @ hT[f]  ([128d, C] PSUM chains
       over 16 f-tiles) — transposed output avoids re-tiling hT and keeps the
       free dim at C; the host transposes yT back during the combine.

Weights are host-swizzled to bf16 so every weight DMA moves contiguous rows,
and all per-rep weight traffic (12 MB) streams behind the ~88 us of PE work.
"""

import os

import numpy as np

import concourse.bass as bass  # noqa: F401  (kept for parity with tooling)
import concourse.mybir as mybir
import concourse.tile as tile
from concourse import bacc
from concourse.bass_utils import run_bass_kernel_spmd

f32 = mybir.dt.float32
bf16 = mybir.dt.bfloat16
AF = mybir.ActivationFunctionType
OP = mybir.AluOpType

np_bf16 = mybir.dt.np(bf16)

_REPS = int(os.environ.get("MOE_REPS", "1"))

P = 128
T = 2048
D = 1024
F = 2048
E = 8
ND = D // P  # 8 d tiles
NF = F // P  # 16 f tiles


def _chunks(C):
    """Split [0, C) into balanced PSUM-bank-sized (<=512) column chunks.

    Balanced widths keep every matmul's moving pass longer than the 128-cycle
    stationary load, so back-to-back weight loads can hide behind compute.
    """
    n = (C + 511) // 512
    base = C // n
    rem = C - base * n
    out, off = [], 0
    for i in range(n):
        w = base + (1 if i < rem else 0)
        out.append((off, w))
        off += w
    return out


def build_moe(C, reps=None, mode="full"):
    """mode: "full" | "nodma" (weights resident, loaded once before the rep
    loop) | "dmaonly" (weight streams only, no compute) — the latter two are
    profiling aids for isolating PE vs DMA limits."""
    global _REPS
    if reps is not None:
        _REPS = reps
    CHS = _chunks(C)

    nc = bacc.Bacc("TRN2", target_bir_lowering=False, debug=False)

    # xc d-tiles packed partition-major: [p, dt, c] = xc[c, dt*128+p]
    xcs = nc.dram_tensor("xcs", [P, ND, C], bf16, kind="ExternalInput").ap()
    # w1/v1 swizzles packed in f-tile pairs: one 8KB-row DMA per 2 f-tiles
    wv1s = nc.dram_tensor("wv1s", [NF // 2, P, 4 * ND * P], bf16,
                          kind="ExternalInput").ap()
    # all w2 tiles packed partition-major: [p, f, d] = w2[f*128+p, d]
    w2s = nc.dram_tensor("w2s", [P, NF, D], bf16, kind="ExternalInput").ap()
    yT = nc.dram_tensor("yT", [D, C], f32, kind="ExternalOutput").ap()

    with tile.TileContext(nc) as tc:
        with (
            tc.tile_pool(name="xct", bufs=4) as xctpool,
            tc.tile_pool(name="w12",
                         bufs=(NF // 2 if mode == "nodma" else 3)) as wpool,
            tc.tile_pool(name="w2p",
                         bufs=(1 if mode == "nodma" else 2)) as w2pool,
            tc.tile_pool(name="ht", bufs=2 * NF) as htpool,
            tc.tile_pool(name="ssb", bufs=4) as spool,
            tc.tile_pool(name="ysb", bufs=3) as ypool,
            tc.tile_pool(name="psum", bufs=1, space="PSUM") as psp,
        ):
            res_w = {}

            def _load_wv(g):
                # f-tiles 2g and 2g+1 in one DMA (SP hwdge queue)
                wv_sb = wpool.tile([P, 4 * ND * P], bf16, tag="w12",
                                   name=f"wv1_{g}")
                nc.sync.dma_start(out=wv_sb[:], in_=wv1s[g, :, :])
                return wv_sb

            def _load_w2_tile():
                return w2pool.tile([P, NF, D], bf16, tag="w2", name="w2")

            def _load_w2_quarter(w2_sb, q):
                # W2 in 4 spread DMAs on the SWDGE queue (Pool engine), so
                # the stream rides a third DMA path and never blocks wv1
                nc.gpsimd.dma_start(
                    out=w2_sb[:, 4 * q:4 * q + 4, :],
                    in_=w2s[:, 4 * q:4 * q + 4, :],
                )

            def _load_w2():
                w2_sb = _load_w2_tile()
                for q in range(4):
                    _load_w2_quarter(w2_sb, q)
                return w2_sb

            if mode == "nodma":
                for g in range(NF // 2):
                    res_w[g] = _load_wv(g)
                res_w["w2"] = _load_w2()

            def _emit_dmaonly():
                for g in range(NF // 2):
                    _load_wv(g)
                _load_w2()

            def _emit_body():
                # compact token activations, packed [p, dt, c]; two tiles
                # (one DMA each) so the first M12 chain starts after half
                # the transfer. ACT hwdge queue: runs in parallel with the
                # w1/v1 weight stream on the SP queue.
                xc_h = [None, None]
                for q in range(2):
                    xc_h[q] = xctpool.tile([P, ND // 2, C], bf16, tag="xct",
                                           name=f"xcs_{q}")
                    nc.scalar.dma_start(
                        out=xc_h[q][:], in_=xcs[:, 4 * q:4 * q + 4, :]
                    )
                xc_sb = [xc_h[d // 4][:, d % 4, :] for d in range(ND)]

                # ---- M12: hT[f] = silu(W1 xc) * (V1 xc), f-major ----
                hT = [None] * NF
                w2_sb = res_w["w2"] if mode == "nodma" else _load_w2_tile()
                for f in range(NF):
                    hT[f] = htpool.tile([P, C], bf16, tag="ht", name=f"ht_{f}")
                    if mode != "nodma" and f % 4 == 2:
                        _load_w2_quarter(w2_sb, f // 4)
                    if f % 2 == 0:
                        wv_sb = (res_w[f // 2] if mode == "nodma"
                                 else _load_wv(f // 2))
                    half = (f % 2) * 2 * ND * P
                    w1_sb = wv_sb[:, half:half + ND * P]
                    v1_sb = wv_sb[:, half + ND * P:half + 2 * ND * P]
                    # d-outer, chunks paired under one stationary load so the
                    # PE loads each 128x128 weight block once per chain
                    a_ps = [psp.tile([P, 512], f32, tag="mm", bufs=4,
                                     name=f"a_ps_{ci}") for ci in range(len(CHS))]
                    for d in range(ND):
                        for ci, (off, w) in enumerate(CHS):
                            nc.tensor.matmul(
                                out=a_ps[ci][:, :w],
                                lhsT=w1_sb[:, d * P:(d + 1) * P],
                                rhs=xc_sb[d][:, off:off + w],
                                start=(d == 0), stop=(d == ND - 1),
                            )
                    b_ps = [psp.tile([P, 512], f32, tag="mm", bufs=4,
                                     name=f"b_ps_{ci}") for ci in range(len(CHS))]
                    for d in range(ND):
                        for ci, (off, w) in enumerate(CHS):
                            nc.tensor.matmul(
                                out=b_ps[ci][:, :w],
                                lhsT=v1_sb[:, d * P:(d + 1) * P],
                                rhs=xc_sb[d][:, off:off + w],
                                start=(d == 0), stop=(d == ND - 1),
                            )
                    for ci, (off, w) in enumerate(CHS):
                        s_sb = spool.tile([P, 512], f32, tag="ssb")
                        nc.scalar.activation(s_sb[:, :w], a_ps[ci][:, :w],
                                             AF.Sigmoid)
                        nc.vector.tensor_tensor(
                            out=s_sb[:, :w], in0=s_sb[:, :w],
                            in1=a_ps[ci][:, :w], op=OP.mult,
                        )
                        nc.vector.tensor_tensor(
                            out=hT[f][:, off:off + w], in0=s_sb[:, :w],
                            in1=b_ps[ci][:, :w], op=OP.mult,
                        )

                # ---- M3T: yT[d] = sum_f w2[f, d-block]^T chains @ hT[f] ----
                for d in range(ND):
                    y_sb = ypool.tile([P, C], f32, tag="ysb", name=f"y_{d}")
                    y_ps = [psp.tile([P, 512], f32, tag="y", bufs=4,
                                     name=f"y_ps_{ci}") for ci in range(len(CHS))]
                    for f in range(NF):
                        for ci, (off, w) in enumerate(CHS):
                            nc.tensor.matmul(
                                out=y_ps[ci][:, :w],
                                lhsT=w2_sb[:, f, d * P:(d + 1) * P],
                                rhs=hT[f][:, off:off + w],
                                start=(f == 0), stop=(f == NF - 1),
                            )
                    for ci, (off, w) in enumerate(CHS):
                        nc.scalar.activation(
                            y_sb[:, off:off + w], y_ps[ci][:, :w], AF.Copy
                        )
                    nc.scalar.dma_start(
                        out=yT[d * P:(d + 1) * P, :], in_=y_sb[:]
                    )

            for _rep in range(_REPS):
                if mode == "dmaonly":
                    _emit_dmaonly()
                else:
                    _emit_body()

    return nc


_NC_CACHE = {}


def _get_nc(C, reps=None, mode="full"):
    key = (C, reps if reps is not None else _REPS, mode)
    if key not in _NC_CACHE:
        nc = build_moe(C, reps=reps, mode=mode)
        nc.compile()
        _NC_CACHE[key] = nc
    return _NC_CACHE[key]


def _route(x, gate_w):
    """Host top-2 routing. Returns per-expert (token idx, combine coef)."""
    logits = x.astype(np.float32) @ gate_w.astype(np.float32).T  # [T, E]
    t = np.arange(logits.shape[0])
    sel1 = np.argmax(logits, axis=1)
    l1 = logits[t, sel1]
    masked = logits.copy()
    masked[t, sel1] = -np.inf
    sel2 = np.argmax(masked, axis=1)
    l2 = logits[t, sel2]
    # softmax top-2, L1-renormalized == pairwise sigmoid of the logit gap
    w1c = 1.0 / (1.0 + np.exp(l2 - l1))
    w2c = 1.0 - w1c
    idx, cf = [], []
    for e in range(E):
        m1 = sel1 == e
        m2 = sel2 == e
        ide = np.nonzero(m1 | m2)[0]
        ce = np.where(m1[ide], w1c[ide], w2c[ide]).astype(np.float32)
        idx.append(ide)
        cf.append(ce)
    return idx, cf


def _swizzle_w1(w):
    """(F, D) -> [NF, 128, ND*128] with [f, p, dt*128+fc] = w[f*128+fc, dt*128+p]."""
    v = w.reshape(NF, P, ND, P)  # [f, fc, dt, p]
    return np.ascontiguousarray(v.transpose(0, 3, 2, 1).reshape(NF, P, ND * P))


def _build_in_maps(x, gate_w, w1, v1, w2, C, idx):
    x = np.asarray(x, dtype=np.float32)
    in_maps = []
    for c in range(E):
        ide = idx[c]
        xc = np.zeros((C, D), dtype=np_bf16)
        xc[:len(ide)] = x[ide].astype(np_bf16)
        # [p, dt, c] = xc[c, dt*128+p]
        xcs = np.ascontiguousarray(
            xc.T.reshape(ND, P, C).transpose(1, 0, 2))
        # f-tile pairs of (w1 swizzle | v1 swizzle) side by side
        w1z = _swizzle_w1(np.asarray(w1[c], np.float32).astype(np_bf16))
        v1z = _swizzle_w1(np.asarray(v1[c], np.float32).astype(np_bf16))
        wv = np.concatenate(
            [w1z[0::2], v1z[0::2], w1z[1::2], v1z[1::2]], axis=-1)
        wv = np.ascontiguousarray(wv)
        # [p, f, d] = w2[f*128+p, d]
        w2z = np.ascontiguousarray(
            np.asarray(w2[c], np.float32).astype(np_bf16)
            .reshape(NF, P, D).transpose(1, 0, 2))
        in_maps.append({"xcs": xcs, "wv1s": wv, "w2s": w2z})
    return in_maps


def _capacity(idx):
    C = max(len(i) for i in idx)
    return max(16, (C + 3) // 4 * 4)  # 4-align DMA rows


def kernel(x, gate_w, w1, v1, w2):
    idx, cf = _route(x, gate_w)
    C = _capacity(idx)
    nc = _get_nc(C)
    in_maps = _build_in_maps(x, gate_w, w1, v1, w2, C, idx)
    res = run_bass_kernel_spmd(nc, in_maps, core_ids=list(range(E)))
    out = np.zeros((T, D), dtype=np.float32)
    for c, r in enumerate(res.results):
        n = len(idx[c])
        y = r["yT"].T[:n]  # [n, D] unscaled expert output
        out[idx[c]] += cf[c][:, None] * y
    return out


# revision 34
# speedup vs baseline: 1.0029x; 1.0029x over previous
"""TRN2 Bass/Tile kernel for nn_BlockSparseMoE (T=2048, D=1024, F=2048, E=8, top-2).

Expert parallelism across the 8 NeuronCores: core c owns expert c. The host
performs routing (top-2 of an [T, E] logit matmul — microseconds of numpy) and
the expert-parallel all-to-all dispatch/combine: it gathers each expert's
tokens into a compact d-major activation block xcT = x[idx_e].T, and after the
device run scatters coef * y back into the full [T, D] output.

The device NEFF is a pure fused GLU FFN per expert, in bf16 (fp32 PSUM
accumulate), sized to the actual max expert load C (548 here):

  M12  a = W1 @ xc, b = V1 @ xc  (f-major [128f, C] PSUM chains over 8
       d-tiles; the two <=512-wide C-chunks are paired under each stationary
       128x128 block so the PE loads it once)
       hT[f] = sigmoid(a) * a * b  (ACT sigmoid + 2 DVE mults, bf16 out)
  M3T  yT[d] = sum_f W2[f, d-block]^T-chain @ hT[f]  ([128d, C] PSUM chains
       over 16 f-tiles) — transposed output avoids re-tiling hT and keeps the
       free dim at C (70k cycles vs 82k token-major); the host transposes yT
       back during the combine.

PE floor: 384*C cycles ~= 210k (~71 us at the measured ~2.96 GHz PE clock);
ACT/DVE/DMA all stay below it. Weight traffic (12 MB bf16/rep) is batched
into 13 large DMAs spread over three queues (SP hwdge: w1|v1 f-tile pairs;
ACT hwdge: activations in / out; Pool SWDGE: w2 quarters) so streaming
interference with the PE's SBUF reads is minimized.
"""

import os

import numpy as np

import concourse.bass as bass  # noqa: F401  (kept for parity with tooling)
import concourse.mybir as mybir
import concourse.tile as tile
from concourse import bacc
from concourse.bass_utils import run_bass_kernel_spmd

f32 = mybir.dt.float32
bf16 = mybir.dt.bfloat16
AF = mybir.ActivationFunctionType
OP = mybir.AluOpType

np_bf16 = mybir.dt.np(bf16)

_REPS = int(os.environ.get("MOE_REPS", "1"))

P = 128
T = 2048
D = 1024
F = 2048
E = 8
ND = D // P  # 8 d tiles
NF = F // P  # 16 f tiles


def _chunks(C):
    """Split [0, C) into balanced PSUM-bank-sized (<=512) column chunks.

    Balanced widths keep every matmul's moving pass longer than the 128-cycle
    stationary load, so back-to-back weight loads can hide behind compute.
    """
    n = (C + 511) // 512
    base = C // n
    rem = C - base * n
    out, off = [], 0
    for i in range(n):
        w = base + (1 if i < rem else 0)
        out.append((off, w))
        off += w
    return out


def build_moe(C, reps=None, mode="full"):
    """mode: "full" | "nodma" (weights resident, loaded once before the rep
    loop) | "dmaonly" (weight streams only, no compute) — the latter two are
    profiling aids for isolating PE vs DMA limits."""
    global _REPS
    if reps is not None:
        _REPS = reps
    CHS = _chunks(C)
    # chunk groups of <=2 bound live PSUM tiles per chain at 4 of 8 banks
    CHG = [CHS[i:i + 2] for i in range(0, len(CHS), 2)]

    nc = bacc.Bacc("TRN2", target_bir_lowering=False, debug=False)

    # xc d-tiles packed partition-major: [p, dt, c] = xc[c, dt*128+p]
    xcs = nc.dram_tensor("xcs", [P, ND, C], bf16, kind="ExternalInput").ap()
    # w1/v1 swizzles packed in f-tile pairs: one 8KB-row DMA per 2 f-tiles
    wv1s = nc.dram_tensor("wv1s", [NF // 2, P, 4 * ND * P], bf16,
                          kind="ExternalInput").ap()
    # all w2 tiles packed partition-major: [p, f, d] = w2[f*128+p, d]
    w2s = nc.dram_tensor("w2s", [P, NF, D], bf16, kind="ExternalInput").ap()
    yT = nc.dram_tensor("yT", [D, C], f32, kind="ExternalOutput").ap()

    big = C > 768  # shrink double-buffering to fit SBUF at large capacity
    with tile.TileContext(nc) as tc:
        with (
            tc.tile_pool(name="xct", bufs=(2 if big else 4)) as xctpool,
            tc.tile_pool(name="w12",
                         bufs=(NF // 2 if mode == "nodma" else 3)) as wpool,
            tc.tile_pool(name="w2p",
                         bufs=(1 if mode == "nodma" else 2)) as w2pool,
            tc.tile_pool(name="ht",
                         bufs=(NF + 4 if big else 2 * NF)) as htpool,
            tc.tile_pool(name="ssb", bufs=4) as spool,
            tc.tile_pool(name="ysb", bufs=(2 if big else 3)) as ypool,
            tc.tile_pool(name="psum", bufs=1, space="PSUM") as psp,
        ):
            res_w = {}

            def _load_wv(g):
                # f-tiles 2g and 2g+1 in one DMA (SP hwdge queue)
                wv_sb = wpool.tile([P, 4 * ND * P], bf16, tag="w12",
                                   name=f"wv1_{g}")
                nc.sync.dma_start(out=wv_sb[:], in_=wv1s[g, :, :])
                return wv_sb

            def _load_w2_tile():
                return w2pool.tile([P, NF, D], bf16, tag="w2", name="w2")

            def _load_w2_quarter(w2_sb, q):
                # W2 in 4 spread DMAs on the SWDGE queue (Pool engine), so
                # the stream rides a third DMA path and never blocks wv1
                eng = nc.scalar if mode == "w2act" else nc.gpsimd
                eng.dma_start(
                    out=w2_sb[:, 4 * q:4 * q + 4, :],
                    in_=w2s[:, 4 * q:4 * q + 4, :],
                )

            def _load_w2():
                w2_sb = _load_w2_tile()
                for q in range(4):
                    _load_w2_quarter(w2_sb, q)
                return w2_sb

            if mode == "nodma":
                for g in range(NF // 2):
                    res_w[g] = _load_wv(g)
                res_w["w2"] = _load_w2()

            def _emit_dmaonly():
                for g in range(NF // 2):
                    _load_wv(g)
                _load_w2()

            def _emit_body():
                # compact token activations, packed [p, dt, c]; two tiles
                # (one DMA each) so the first M12 chain starts after half
                # the transfer. ACT hwdge queue: runs in parallel with the
                # w1/v1 weight stream on the SP queue.
                xc_h = [None, None]
                for q in range(2):
                    xc_h[q] = xctpool.tile([P, ND // 2, C], bf16, tag="xct",
                                           name=f"xcs_{q}")
                    nc.scalar.dma_start(
                        out=xc_h[q][:], in_=xcs[:, 4 * q:4 * q + 4, :]
                    )
                xc_sb = [xc_h[d // 4][:, d % 4, :] for d in range(ND)]

                # ---- M12: hT[f] = silu(W1 xc) * (V1 xc), f-major ----
                hT = [None] * NF
                w2_sb = res_w["w2"] if mode == "nodma" else _load_w2_tile()
                for f in range(NF):
                    hT[f] = htpool.tile([P, C], bf16, tag="ht", name=f"ht_{f}")
                    if mode != "nodma" and f % 4 == 2:
                        _load_w2_quarter(w2_sb, f // 4)
                    if f % 2 == 0:
                        wv_sb = (res_w[f // 2] if mode == "nodma"
                                 else _load_wv(f // 2))
                    half = (f % 2) * 2 * ND * P
                    w1_sb = wv_sb[:, half:half + ND * P]
                    v1_sb = wv_sb[:, half + ND * P:half + 2 * ND * P]
                    # d-outer, chunks paired under one stationary load so the
                    # PE loads each 128x128 weight block once per chain
                    for grp in CHG:
                        a_ps = [psp.tile([P, 512], f32, tag="mm", bufs=4,
                                         name=f"a_ps_{ci}")
                                for ci in range(len(grp))]
                        for d in range(ND):
                            for ci, (off, w) in enumerate(grp):
                                nc.tensor.matmul(
                                    out=a_ps[ci][:, :w],
                                    lhsT=w1_sb[:, d * P:(d + 1) * P],
                                    rhs=xc_sb[d][:, off:off + w],
                                    start=(d == 0), stop=(d == ND - 1),
                                )
                        b_ps = [psp.tile([P, 512], f32, tag="mm", bufs=4,
                                         name=f"b_ps_{ci}")
                                for ci in range(len(grp))]
                        for d in range(ND):
                            for ci, (off, w) in enumerate(grp):
                                nc.tensor.matmul(
                                    out=b_ps[ci][:, :w],
                                    lhsT=v1_sb[:, d * P:(d + 1) * P],
                                    rhs=xc_sb[d][:, off:off + w],
                                    start=(d == 0), stop=(d == ND - 1),
                                )
                        for ci, (off, w) in enumerate(grp):
                            s_sb = spool.tile([P, 512], f32, tag="ssb")
                            nc.scalar.activation(s_sb[:, :w], a_ps[ci][:, :w],
                                                 AF.Sigmoid)
                            nc.vector.tensor_tensor(
                                out=s_sb[:, :w], in0=s_sb[:, :w],
                                in1=a_ps[ci][:, :w], op=OP.mult,
                            )
                            nc.vector.tensor_tensor(
                                out=hT[f][:, off:off + w], in0=s_sb[:, :w],
                                in1=b_ps[ci][:, :w], op=OP.mult,
                            )

                # ---- M3T: yT[d] = sum_f w2[f, d-block]^T chains @ hT[f] ----
                for d in range(ND):
                    y_sb = ypool.tile([P, C], f32, tag="ysb", name=f"y_{d}")
                    for grp in CHG:
                        y_ps = [psp.tile([P, 512], f32, tag="y", bufs=4,
                                         name=f"y_ps_{ci}")
                                for ci in range(len(grp))]
                        for f in range(NF):
                            for ci, (off, w) in enumerate(grp):
                                nc.tensor.matmul(
                                    out=y_ps[ci][:, :w],
                                    lhsT=w2_sb[:, f, d * P:(d + 1) * P],
                                    rhs=hT[f][:, off:off + w],
                                    start=(f == 0), stop=(f == NF - 1),
                                )
                        for ci, (off, w) in enumerate(grp):
                            nc.scalar.activation(
                                y_sb[:, off:off + w], y_ps[ci][:, :w], AF.Copy
                            )
                    nc.scalar.dma_start(
                        out=yT[d * P:(d + 1) * P, :], in_=y_sb[:]
                    )

            for _rep in range(_REPS):
                if mode == "dmaonly":
                    _emit_dmaonly()
                else:
                    _emit_body()

    return nc


_NC_CACHE = {}


def _get_nc(C, reps=None, mode="full"):
    key = (C, reps if reps is not None else _REPS, mode)
    if key not in _NC_CACHE:
        nc = build_moe(C, reps=reps, mode=mode)
        nc.compile()
        _NC_CACHE[key] = nc
    return _NC_CACHE[key]


def _route(x, gate_w):
    """Host top-2 routing. Returns per-expert (token idx, combine coef)."""
    logits = x.astype(np.float32) @ gate_w.astype(np.float32).T  # [T, E]
    t = np.arange(logits.shape[0])
    sel1 = np.argmax(logits, axis=1)
    l1 = logits[t, sel1]
    masked = logits.copy()
    masked[t, sel1] = -np.inf
    sel2 = np.argmax(masked, axis=1)
    l2 = logits[t, sel2]
    # softmax top-2, L1-renormalized == pairwise sigmoid of the logit gap
    w1c = 1.0 / (1.0 + np.exp(l2 - l1))
    w2c = 1.0 - w1c
    idx, cf = [], []
    for e in range(E):
        m1 = sel1 == e
        m2 = sel2 == e
        ide = np.nonzero(m1 | m2)[0]
        ce = np.where(m1[ide], w1c[ide], w2c[ide]).astype(np.float32)
        idx.append(ide)
        cf.append(ce)
    return idx, cf


def _swizzle_w1(w):
    """(F, D) -> [NF, 128, ND*128] with [f, p, dt*128+fc] = w[f*128+fc, dt*128+p]."""
    v = w.reshape(NF, P, ND, P)  # [f, fc, dt, p]
    return np.ascontiguousarray(v.transpose(0, 3, 2, 1).reshape(NF, P, ND * P))


def _build_in_maps(x, gate_w, w1, v1, w2, C, idx):
    x = np.asarray(x, dtype=np.float32)
    in_maps = []
    for c in range(E):
        ide = idx[c]
        xc = np.zeros((C, D), dtype=np_bf16)
        xc[:len(ide)] = x[ide].astype(np_bf16)
        # [p, dt, c] = xc[c, dt*128+p]
        xcs = np.ascontiguousarray(
            xc.T.reshape(ND, P, C).transpose(1, 0, 2))
        # f-tile pairs of (w1 swizzle | v1 swizzle) side by side
        w1z = _swizzle_w1(np.asarray(w1[c], np.float32).astype(np_bf16))
        v1z = _swizzle_w1(np.asarray(v1[c], np.float32).astype(np_bf16))
        wv = np.concatenate(
            [w1z[0::2], v1z[0::2], w1z[1::2], v1z[1::2]], axis=-1)
        wv = np.ascontiguousarray(wv)
        # [p, f, d] = w2[f*128+p, d]
        w2z = np.ascontiguousarray(
            np.asarray(w2[c], np.float32).astype(np_bf16)
            .reshape(NF, P, D).transpose(1, 0, 2))
        in_maps.append({"xcs": xcs, "wv1s": wv, "w2s": w2z})
    return in_maps


def _capacity(idx):
    C = max(len(i) for i in idx)
    return max(16, (C + 3) // 4 * 4)  # 4-align DMA rows


def kernel(x, gate_w, w1, v1, w2):
    idx, cf = _route(x, gate_w)
    C = _capacity(idx)
    nc = _get_nc(C)
    in_maps = _build_in_maps(x, gate_w, w1, v1, w2, C, idx)
    res = run_bass_kernel_spmd(nc, in_maps, core_ids=list(range(E)))
    out = np.zeros((T, D), dtype=np.float32)
    for c, r in enumerate(res.results):
        n = len(idx[c])
        y = r["yT"].T[:n]  # [n, D] unscaled expert output
        out[idx[c]] += cf[c][:, None] * y
    return out


# revision 35
# speedup vs baseline: 1.0487x; 1.0457x over previous
"""TRN2 Bass/Tile kernel for nn_BlockSparseMoE (T=2048, D=1024, F=2048, E=8, top-2).

Expert parallelism across the 8 NeuronCores: core c owns expert c. The host
performs routing (top-2 of an [T, E] logit matmul — microseconds of numpy) and
the expert-parallel all-to-all dispatch/combine: it gathers each expert's
tokens into a compact d-major activation block xcT = x[idx_e].T, and after the
device run scatters coef * y back into the full [T, D] output.

The device NEFF is a pure fused GLU FFN per expert, in bf16 (fp32 PSUM
accumulate), sized to the actual max expert load C (548 here):

  M12  a = W1 @ xc, b = V1 @ xc  (f-major [128f, C] PSUM chains over 8
       d-tiles; the two <=512-wide C-chunks are paired under each stationary
       128x128 block so the PE loads it once)
       hT[f] = sigmoid(a) * a * b  (ACT sigmoid + 2 DVE mults, bf16 out)
  M3T  yT[d] = sum_f W2[f, d-block]^T-chain @ hT[f]  ([128d, C] PSUM chains
       over 16 f-tiles) — transposed output avoids re-tiling hT and keeps the
       free dim at C (70k cycles vs 82k token-major); the host transposes yT
       back during the combine.

PE floor: 384*C cycles ~= 210k (~71 us at the measured ~2.96 GHz PE clock);
ACT/DVE/DMA all stay below it. Weight traffic (12 MB bf16/rep) is batched
into 13 large DMAs spread over three queues (SP hwdge: w1|v1 f-tile pairs;
ACT hwdge: activations in / out; Pool SWDGE: w2 quarters) so streaming
interference with the PE's SBUF reads is minimized.
"""

import os

import numpy as np

import concourse.bass as bass  # noqa: F401  (kept for parity with tooling)
import concourse.mybir as mybir
import concourse.tile as tile
from concourse import bacc
from concourse.bass_utils import run_bass_kernel_spmd

f32 = mybir.dt.float32
bf16 = mybir.dt.bfloat16
AF = mybir.ActivationFunctionType
OP = mybir.AluOpType

np_bf16 = mybir.dt.np(bf16)

_REPS = int(os.environ.get("MOE_REPS", "1"))

P = 128
T = 2048
D = 1024
F = 2048
E = 8
ND = D // P  # 8 d tiles
NF = F // P  # 16 f tiles


def _chunks(C):
    """Split [0, C) into balanced PSUM-bank-sized (<=512) column chunks.

    Balanced widths keep every matmul's moving pass longer than the 128-cycle
    stationary load, so back-to-back weight loads can hide behind compute.
    """
    n = (C + 511) // 512
    base = C // n
    rem = C - base * n
    out, off = [], 0
    for i in range(n):
        w = base + (1 if i < rem else 0)
        out.append((off, w))
        off += w
    return out


def build_moe(C, reps=None, mode="full"):
    """mode: "full" | "nodma" (weights resident, loaded once before the rep
    loop) | "dmaonly" (weight streams only, no compute) — the latter two are
    profiling aids for isolating PE vs DMA limits."""
    global _REPS
    if reps is not None:
        _REPS = reps
    CHS = _chunks(C)
    # chunk groups of <=2 bound live PSUM tiles per chain at 4 of 8 banks
    CHG = [CHS[i:i + 2] for i in range(0, len(CHS), 2)]

    nc = bacc.Bacc("TRN2", target_bir_lowering=False, debug=False)

    # xc d-tiles packed partition-major: [p, dt, c] = xc[c, dt*128+p]
    xcs = nc.dram_tensor("xcs", [P, ND, C], bf16, kind="ExternalInput").ap()
    # w1/v1 swizzles packed in f-tile pairs: one 8KB-row DMA per 2 f-tiles
    wv1s = nc.dram_tensor("wv1s", [NF // 2, P, 4 * ND * P], bf16,
                          kind="ExternalInput").ap()
    # all w2 tiles packed partition-major: [p, f, d] = w2[f*128+p, d]
    w2s = nc.dram_tensor("w2s", [P, NF, D], bf16, kind="ExternalInput").ap()
    yT = nc.dram_tensor("yT", [D, C], f32, kind="ExternalOutput").ap()

    big = C > 768  # shrink double-buffering to fit SBUF at large capacity
    with tile.TileContext(nc) as tc:
        with (
            tc.tile_pool(name="xct", bufs=(2 if big else 4)) as xctpool,
            tc.tile_pool(name="w12",
                         bufs=(NF // 2 if mode == "nodma" else 3)) as wpool,
            tc.tile_pool(name="w2p",
                         bufs=(1 if mode == "nodma" else 2)) as w2pool,
            tc.tile_pool(name="ht",
                         bufs=(NF + 4 if big else 2 * NF)) as htpool,
            tc.tile_pool(name="ssb", bufs=4) as spool,
            tc.tile_pool(name="ysb", bufs=(2 if big else 3)) as ypool,
            tc.tile_pool(name="psum", bufs=1, space="PSUM") as psp,
        ):
            res_w = {}

            def _load_wv(g):
                # f-tiles 2g and 2g+1 in one DMA (SP hwdge queue)
                wv_sb = wpool.tile([P, 4 * ND * P], bf16, tag="w12",
                                   name=f"wv1_{g}")
                if mode == "wvsplit":
                    half = 2 * ND * P
                    nc.sync.dma_start(out=wv_sb[:, :half],
                                      in_=wv1s[g, :, :half])
                    nc.sync.dma_start(out=wv_sb[:, half:],
                                      in_=wv1s[g, :, half:])
                else:
                    nc.sync.dma_start(out=wv_sb[:], in_=wv1s[g, :, :])
                return wv_sb

            def _load_w2_tile():
                return w2pool.tile([P, NF, D], bf16, tag="w2", name="w2")

            def _load_w2_quarter(w2_sb, q):
                # W2 in 4 spread DMAs on the SWDGE queue (Pool engine), so
                # the stream rides a third DMA path and never blocks wv1
                eng = nc.scalar if mode == "w2act" else nc.gpsimd
                eng.dma_start(
                    out=w2_sb[:, 4 * q:4 * q + 4, :],
                    in_=w2s[:, 4 * q:4 * q + 4, :],
                )

            def _load_w2():
                w2_sb = _load_w2_tile()
                for q in range(4):
                    _load_w2_quarter(w2_sb, q)
                return w2_sb

            if mode == "nodma":
                for g in range(NF // 2):
                    res_w[g] = _load_wv(g)
                res_w["w2"] = _load_w2()

            def _emit_dmaonly():
                for g in range(NF // 2):
                    _load_wv(g)
                _load_w2()

            def _emit_body():
                # compact token activations, packed [p, dt, c]; two tiles
                # (one DMA each) so the first M12 chain starts after half
                # the transfer. ACT hwdge queue: runs in parallel with the
                # w1/v1 weight stream on the SP queue.
                xc_h = [None, None]
                for q in range(2):
                    xc_h[q] = xctpool.tile([P, ND // 2, C], bf16, tag="xct",
                                           name=f"xcs_{q}")
                    nc.scalar.dma_start(
                        out=xc_h[q][:], in_=xcs[:, 4 * q:4 * q + 4, :]
                    )
                xc_sb = [xc_h[d // 4][:, d % 4, :] for d in range(ND)]

                # ---- M12: hT[f] = silu(W1 xc) * (V1 xc), f-major ----
                hT = [None] * NF
                w2_sb = res_w["w2"] if mode == "nodma" else _load_w2_tile()
                for f in range(NF):
                    hT[f] = htpool.tile([P, C], bf16, tag="ht", name=f"ht_{f}")
                    if mode != "nodma" and f % 4 == 2:
                        _load_w2_quarter(w2_sb, f // 4)
                    if f % 2 == 0:
                        wv_sb = (res_w[f // 2] if mode == "nodma"
                                 else _load_wv(f // 2))
                    half = (f % 2) * 2 * ND * P
                    w1_sb = wv_sb[:, half:half + ND * P]
                    v1_sb = wv_sb[:, half + ND * P:half + 2 * ND * P]
                    # d-outer, chunks paired under one stationary load so the
                    # PE loads each 128x128 weight block once per chain
                    for grp in CHG:
                        a_ps = [psp.tile([P, 512], f32, tag="mm", bufs=4,
                                         name=f"a_ps_{ci}")
                                for ci in range(len(grp))]
                        for d in range(ND):
                            for ci, (off, w) in enumerate(grp):
                                nc.tensor.matmul(
                                    out=a_ps[ci][:, :w],
                                    lhsT=w1_sb[:, d * P:(d + 1) * P],
                                    rhs=xc_sb[d][:, off:off + w],
                                    start=(d == 0), stop=(d == ND - 1),
                                )
                        b_ps = [psp.tile([P, 512], f32, tag="mm", bufs=4,
                                         name=f"b_ps_{ci}")
                                for ci in range(len(grp))]
                        for d in range(ND):
                            for ci, (off, w) in enumerate(grp):
                                nc.tensor.matmul(
                                    out=b_ps[ci][:, :w],
                                    lhsT=v1_sb[:, d * P:(d + 1) * P],
                                    rhs=xc_sb[d][:, off:off + w],
                                    start=(d == 0), stop=(d == ND - 1),
                                )
                        for ci, (off, w) in enumerate(grp):
                            s_sb = spool.tile([P, 512], f32, tag="ssb")
                            nc.scalar.activation(s_sb[:, :w], a_ps[ci][:, :w],
                                                 AF.Sigmoid)
                            nc.vector.tensor_tensor(
                                out=s_sb[:, :w], in0=s_sb[:, :w],
                                in1=a_ps[ci][:, :w], op=OP.mult,
                            )
                            nc.vector.tensor_tensor(
                                out=hT[f][:, off:off + w], in0=s_sb[:, :w],
                                in1=b_ps[ci][:, :w], op=OP.mult,
                            )

                # ---- M3T: yT[d] = sum_f w2[f, d-block]^T chains @ hT[f] ----
                for d in range(ND):
                    y_sb = ypool.tile([P, C], f32, tag="ysb", name=f"y_{d}")
                    for grp in CHG:
                        y_ps = [psp.tile([P, 512], f32, tag="y", bufs=4,
                                         name=f"y_ps_{ci}")
                                for ci in range(len(grp))]
                        for f in range(NF):
                            for ci, (off, w) in enumerate(grp):
                                nc.tensor.matmul(
                                    out=y_ps[ci][:, :w],
                                    lhsT=w2_sb[:, f, d * P:(d + 1) * P],
                                    rhs=hT[f][:, off:off + w],
                                    start=(f == 0), stop=(f == NF - 1),
                                )
                        for ci, (off, w) in enumerate(grp):
                            nc.scalar.activation(
                                y_sb[:, off:off + w], y_ps[ci][:, :w], AF.Copy
                            )
                    nc.scalar.dma_start(
                        out=yT[d * P:(d + 1) * P, :], in_=y_sb[:]
                    )

            for _rep in range(_REPS):
                if mode == "dmaonly":
                    _emit_dmaonly()
                else:
                    _emit_body()

    return nc


_NC_CACHE = {}


def _get_nc(C, reps=None, mode="full"):
    key = (C, reps if reps is not None else _REPS, mode)
    if key not in _NC_CACHE:
        nc = build_moe(C, reps=reps, mode=mode)
        nc.compile()
        _NC_CACHE[key] = nc
    return _NC_CACHE[key]


def _route(x, gate_w):
    """Host top-2 routing. Returns per-expert (token idx, combine coef)."""
    logits = x.astype(np.float32) @ gate_w.astype(np.float32).T  # [T, E]
    t = np.arange(logits.shape[0])
    sel1 = np.argmax(logits, axis=1)
    l1 = logits[t, sel1]
    masked = logits.copy()
    masked[t, sel1] = -np.inf
    sel2 = np.argmax(masked, axis=1)
    l2 = logits[t, sel2]
    # softmax top-2, L1-renormalized == pairwise sigmoid of the logit gap
    w1c = 1.0 / (1.0 + np.exp(l2 - l1))
    w2c = 1.0 - w1c
    idx, cf = [], []
    for e in range(E):
        m1 = sel1 == e
        m2 = sel2 == e
        ide = np.nonzero(m1 | m2)[0]
        ce = np.where(m1[ide], w1c[ide], w2c[ide]).astype(np.float32)
        idx.append(ide)
        cf.append(ce)
    return idx, cf


def _swizzle_w1(w):
    """(F, D) -> [NF, 128, ND*128] with [f, p, dt*128+fc] = w[f*128+fc, dt*128+p]."""
    v = w.reshape(NF, P, ND, P)  # [f, fc, dt, p]
    return np.ascontiguousarray(v.transpose(0, 3, 2, 1).reshape(NF, P, ND * P))


def _build_in_maps(x, gate_w, w1, v1, w2, C, idx):
    x = np.asarray(x, dtype=np.float32)
    in_maps = []
    for c in range(E):
        ide = idx[c]
        xc = np.zeros((C, D), dtype=np_bf16)
        xc[:len(ide)] = x[ide].astype(np_bf16)
        # [p, dt, c] = xc[c, dt*128+p]
        xcs = np.ascontiguousarray(
            xc.T.reshape(ND, P, C).transpose(1, 0, 2))
        # f-tile pairs of (w1 swizzle | v1 swizzle) side by side
        w1z = _swizzle_w1(np.asarray(w1[c], np.float32).astype(np_bf16))
        v1z = _swizzle_w1(np.asarray(v1[c], np.float32).astype(np_bf16))
        wv = np.concatenate(
            [w1z[0::2], v1z[0::2], w1z[1::2], v1z[1::2]], axis=-1)
        wv = np.ascontiguousarray(wv)
        # [p, f, d] = w2[f*128+p, d]
        w2z = np.ascontiguousarray(
            np.asarray(w2[c], np.float32).astype(np_bf16)
            .reshape(NF, P, D).transpose(1, 0, 2))
        in_maps.append({"xcs": xcs, "wv1s": wv, "w2s": w2z})
    return in_maps


def _capacity(idx):
    C = max(len(i) for i in idx)
    return max(16, (C + 3) // 4 * 4)  # 4-align DMA rows


def kernel(x, gate_w, w1, v1, w2):
    idx, cf = _route(x, gate_w)
    C = _capacity(idx)
    nc = _get_nc(C)
    in_maps = _build_in_maps(x, gate_w, w1, v1, w2, C, idx)
    res = run_bass_kernel_spmd(nc, in_maps, core_ids=list(range(E)))
    out = np.zeros((T, D), dtype=np.float32)
    for c, r in enumerate(res.results):
        n = len(idx[c])
        y = r["yT"].T[:n]  # [n, D] unscaled expert output
        out[idx[c]] += cf[c][:, None] * y
    return out
